# revision 6
# baseline (speedup 1.0000x reference)
"""Trainium2 Bass kernel for nn_DenSparseMatrix (gnn_message_passing).

Math: out[b, o] = sum_k rm[o,k] * s[idx[o,k], k] * x[b, idx[o,k]],
      s = forward_weights * forward_mask  (elementwise, [I, W])

Strategy (8 NeuronCores, SPMD).  SWDGE descriptor generation on the Pool
engine (one Q7 core pair, ~1.5ns/descriptor + ~1us/instruction) is the
serial bottleneck, so the host works to minimize descriptor count:

  * rm and fm are 0/1-valued; only ~1/4 of the (o, k) tokens have a
    nonzero coefficient c[o,k] = rm[o,k] * s[idx[o,k], k].  Zero tokens
    are dropped host-side.
  * Outputs are assigned to cores by greedy graph coloring so outputs
    sharing an input land on different cores (an input used twice on
    one core forces two separate row fetches).
  * Each core gets its own gather table: a permutation of the x columns
    into 256B pair rows [x[:,ia] | x[:,ib]], paired so that both halves
    of a fetched row usually belong to the SAME output (one descriptor
    then covers two tokens).  Tokens whose input is already placed
    reuse that row.  ~42k descriptors/core vs 262k for the dense case.
  * Within each core outputs are sorted by row count so each 128-output
    block has near-uniform T; block position bi uses the max T over the
    8 cores, so all cores share one SPMD program.
  * Gathers are merged up to a 2048-descriptor cap per instruction
    (larger instructions stall on ring backpressure), rotating over the
    4 SWDGE queues.  DVE applies the per-half coefficients and reduces.
"""

import numpy as np

import concourse.bass as bass
import concourse.bacc as bacc
import concourse.mybir as mybir
from concourse.tile import TileContext
from concourse.bass_utils import run_bass_kernel_spmd
from concourse.library_config import mlp

I = 65536
O = 65536
W = 32
B = 32
NCORES = 8
NROWS = I // 2                # 32768 table rows (int16 gather index limit)
NBLK = (O // NCORES) // 128   # 64 block positions per core
DESC_CAP = 3072               # max descriptors per merged gather
NQ = 4                        # SWDGE queues used round-robin
F32 = mybir.dt.float32
I16 = mybir.dt.int16


def _build_nc(t_list, groups):
    sum_t = sum(t_list)
    t_max = max(t_list)
    cap_t = max(sum(t_list[a:b]) for a, b in groups)
    nc = bacc.Bacc("TRN2", target_bir_lowering=False, debug=False,
                   num_devices=NCORES, num_swdge_queues=NQ,
                   dynamic_dma_scratch_size=65536)

    tab_d = nc.dram_tensor("tab", [NROWS, 2 * B], F32, kind="ExternalInput")
    idx_d = nc.dram_tensor("idx", [128, 8 * sum_t], I16, kind="ExternalInput")
    c01_d = nc.dram_tensor("c01", [128, 2 * sum_t], F32, kind="ExternalInput")
    out_d = nc.dram_tensor("out", [128, NBLK * B], F32, kind="ExternalOutput")

    with TileContext(nc) as tc:
        nc.gpsimd.load_library(mlp)

        with (
            tc.tile_pool(name="pres", bufs=1) as pres,
            tc.tile_pool(name="pg", bufs=6) as pg,
            tc.tile_pool(name="ptmp", bufs=3) as ptmp,
        ):
            idx_all = pres.tile([128, 8 * sum_t], I16)
            nc.sync.dma_start(idx_all[:], idx_d[:])
            c01_all = pres.tile([128, 2 * sum_t], F32)
            nc.sync.dma_start(c01_all[:], c01_d[:])
            ocore = pres.tile([128, NBLK * B], F32)

            goff = [0]
            for a, b in groups:
                goff.append(goff[-1] + sum(t_list[a:b]))

            for g, (a, bnd) in enumerate(groups):
                tg = sum(t_list[a:bnd])
                off = goff[g]
                if tg == 0:
                    for bi in range(a, bnd):
                        nc.vector.memset(
                            ocore[:, bi * B:(bi + 1) * B], 0.0)
                    continue
                G = pg.tile([128, cap_t, 2 * B], F32, tag="G")
                nc.gpsimd.dma_gather(
                    G[:, :tg, :], tab_d[:, :],
                    idx_all[:, 8 * off:8 * (off + tg)],
                    128 * tg, 128 * tg, 2 * B,
                    single_packet=False, queue_num=g % NQ)

                gv = G[:]
                boff = 0
                for bi in range(a, bnd):
                    T = t_list[bi]
                    osl = ocore[:, bi * B:(bi + 1) * B]
                    if T == 0:
                        nc.vector.memset(osl, 0.0)
                        continue
                    cv = c01_all[:, 2 * (off + boff):2 * (off + boff + T)]
                    tmp = ptmp.tile([128, B, 2 * t_max], F32, tag="tmp")
                    tv = tmp[:]
                    # tmp[p, b, u] = G[p, boff*64 + 32u + b] * c01[p, u]
                    gx = bass.AP(gv.tensor, gv.offset + boff * 2 * B,
                                 [list(gv.ap[0]), [B, 2 * T], [1, B]])
                    ab = bass.AP(cv.tensor, cv.offset,
                                 [list(cv.ap[0]), [1, 2 * T], [0, B]])
                    t_ap = bass.AP(tv.tensor, tv.offset,
                                   [list(tv.ap[0]), [1, 2 * T],
                                    [2 * t_max, B]])
                    nc.vector.tensor_mul(t_ap, gx, ab)

                    red_in = bass.AP(tv.tensor, tv.offset,
                                     [list(tv.ap[0]), [2 * t_max, B],
                                      [1, 2 * T]])
                    nc.vector.reduce_sum(osl, red_in,
                                         axis=mybir.AxisListType.X)
                    boff += T

            nc.sync.dma_start(out_d[:], ocore[:])

    nc.compile()
    return nc


def make_plan(x, forward_weights, forward_mask, output_mapping, reverse_mask):
    """Host-side planning: token extraction, core coloring, row pairing."""
    idx = np.asarray(output_mapping).astype(np.int64)
    rm = np.asarray(reverse_mask, dtype=np.float32)
    s = (np.asarray(forward_weights, dtype=np.float32)
         * np.asarray(forward_mask, dtype=np.float32))
    cols = np.arange(W)[None, :]
    c = rm * s[idx, cols]                                  # [O, W]
    nz = c != 0
    cnt = nz.sum(1)
    order = np.argsort(-cnt, kind="stable").tolist()

    # per-output token lists: (input, coeff) with duplicates aggregated
    toks = [None] * O
    for o in range(O):
        k = np.nonzero(nz[o])[0]
        ii = idx[o][k]
        cc = c[o][k]
        if len(ii) != len(set(ii.tolist())):
            agg = {}
            for i, cv in zip(ii.tolist(), cc.tolist()):
                agg[i] = agg.get(i, 0.0) + cv
            toks[o] = list(agg.items())
        else:
            toks[o] = list(zip(ii.tolist(), cc.tolist()))

    # ---- coloring: outputs sharing an input go to different cores
    input_mask = [0] * I
    core_load = [0] * NCORES
    cap = O // NCORES
    core_outputs = [[] for _ in range(NCORES)]
    for o in order:
        forb = 0
        for i, _ in toks[o]:
            forb |= input_mask[i]
        best, bestload = -1, 1 << 30
        for cc_ in range(NCORES):
            if core_load[cc_] >= cap or (forb >> cc_) & 1:
                continue
            if core_load[cc_] < bestload:
                best, bestload = cc_, core_load[cc_]
        if best < 0:
            bestkey = (1 << 30, 1 << 30)
            for cc_ in range(NCORES):
                if core_load[cc_] >= cap:
                    continue
                nconf = sum((input_mask[i] >> cc_) & 1 for i, _ in toks[o])
                key = (nconf, core_load[cc_])
                if key < bestkey:
                    bestkey, best = key, cc_
        core_outputs[best].append(o)
        core_load[best] += 1
        for i, _ in toks[o]:
            input_mask[i] |= 1 << best

    # ---- per-core greedy pairing into 256B rows
    plans = []
    for core in range(NCORES):
        placed = {}            # input -> (row, half)
        row_free = {}          # row -> free half
        nrows = 0
        slots = {}             # output -> list of [row, c0, c1]
        for o in core_outputs[core]:
            free = []
            touched = {}       # row -> [row, c0, c1]
            for i, cv in toks[o]:
                p = placed.get(i)
                if p is None:
                    free.append((i, cv))
                else:
                    sl = touched.get(p[0])
                    if sl is None:
                        sl = touched[p[0]] = [p[0], 0.0, 0.0]
                    sl[1 + p[1]] += cv
            nf = []
            for i, cv in free:
                done = False
                for r in touched:
                    h = row_free.pop(r, None)
                    if h is not None:
                        placed[i] = (r, h)
                        touched[r][1 + h] += cv
                        done = True
                        break
                if not done:
                    nf.append((i, cv))
            free = nf
            for g in range(len(free) // 2):
                (ia, ca), (ib, cb) = free[2 * g], free[2 * g + 1]
                if nrows >= NROWS:
                    raise RuntimeError("row overflow")
                placed[ia] = (nrows, 0)
                placed[ib] = (nrows, 1)
                touched[nrows] = [nrows, ca, cb]
                nrows += 1
            if len(free) % 2:
                i, cv = free[-1]
                r = None
                for rr in row_free:
                    if rr not in touched:
                        r = rr
                        break
                if r is not None:
                    h = row_free.pop(r)
                    placed[i] = (r, h)
                    sl = [r, 0.0, 0.0]
                    sl[1 + h] = cv
                    touched[r] = sl
                else:
                    if nrows >= NROWS:
                        raise RuntimeError("row overflow")
                    placed[i] = (nrows, 0)
                    row_free[nrows] = 1
                    touched[nrows] = [nrows, cv, 0.0]
                    nrows += 1
            slots[o] = list(touched.values())
        # row -> input map for the table
        row_inputs = np.zeros((NROWS, 2), np.int64)
        for i, (r, h) in placed.items():
            row_inputs[r, h] = i
        # sort outputs by slot count desc for uniform blocks
        ordered = sorted(core_outputs[core],
                         key=lambda o: -len(slots[o]))
        plans.append({"slots": slots, "ordered": ordered,
                      "row_inputs": row_inputs})

    # shared t_list across cores
    t_list = []
    for bi in range(NBLK):
        t = 0
        for pl in plans:
            blk = pl["ordered"][bi * 128:(bi + 1) * 128]
            t = max(t, max(len(pl["slots"][o]) for o in blk))
        t_list.append(t)
    t_list = tuple(t_list)

    # merge consecutive blocks into gathers of <= DESC_CAP descriptors
    groups = []
    a = 0
    while a < NBLK:
        b = a + 1
        tg = t_list[a]
        while b < NBLK and (tg + t_list[b]) * 128 <= DESC_CAP:
            tg += t_list[b]
            b += 1
        groups.append((a, b))
        a = b
    groups = tuple(groups)

    return {"plans": plans, "t_list": t_list, "groups": groups}


def make_in_maps(x, plan):
    x = np.asarray(x, dtype=np.float32)
    xT = np.ascontiguousarray(x.T)                         # [I, B]
    t_list = plan["t_list"]

    in_maps = []
    for core in range(NCORES):
        pl = plan["plans"][core]
        slots, ordered, row_inputs = (
            pl["slots"], pl["ordered"], pl["row_inputs"])
        tab = xT[row_inputs.reshape(-1)].reshape(NROWS, 2 * B)
        idx_parts, c_parts = [], []
        for bi, T in enumerate(t_list):
            outs = ordered[bi * 128:(bi + 1) * 128]
            if T == 0:
                continue
            J = np.zeros((128, T), np.int16)
            c01 = np.zeros((128, T, 2), np.float32)
            for p, o in enumerate(outs):
                for s_, (r, c0, c1) in enumerate(slots[o]):
                    J[p, s_] = r
                    c01[p, s_, 0] = c0
                    c01[p, s_, 1] = c1
            # token m = s*128 + p; wrap in 16 partitions, replicate x8
            L = J.T.reshape(8 * T, 16).T                   # [16, 8T]
            idx_parts.append(np.tile(L, (8, 1)))           # [128, 8T]
            c_parts.append(c01.reshape(128, 2 * T))
        in_maps.append({
            "tab": tab,
            "idx": np.ascontiguousarray(np.concatenate(idx_parts, axis=1)),
            "c01": np.ascontiguousarray(np.concatenate(c_parts, axis=1)),
        })
    return in_maps


def unshard_out(results, plan):
    out = np.empty((B, O), np.float32)
    for core in range(NCORES):
        oc = results[core]["out"]                          # [128, NBLK*B]
        vals = oc.reshape(128, NBLK, B).transpose(2, 1, 0) # [B, NBLK, 128]
        perm = np.array(plan["plans"][core]["ordered"])
        out[:, perm] = vals.reshape(B, NBLK * 128)
    return out


_NC = None
_NC_KEY = None


def _get_nc(t_list, groups):
    global _NC, _NC_KEY
    key = (t_list, groups)
    if _NC is None or _NC_KEY != key:
        _NC = _build_nc(t_list, groups)
        _NC_KEY = key
    return _NC


def kernel(x, forward_weights, forward_mask, output_mapping, reverse_mask):
    plan = make_plan(x, forward_weights, forward_mask,
                     output_mapping, reverse_mask)
    nc = _get_nc(plan["t_list"], plan["groups"])
    in_maps = make_in_maps(x, plan)
    res = run_bass_kernel_spmd(nc, in_maps, core_ids=list(range(NCORES)))
    return unshard_out(res.results, plan)


# revision 7
# speedup vs baseline: 1.0953x; 1.0953x over previous
"""Trainium2 Bass kernel for nn_DenSparseMatrix (gnn_message_passing).

Math: out[b, o] = sum_k rm[o,k] * s[idx[o,k], k] * x[b, idx[o,k]],
      s = forward_weights * forward_mask  (elementwise, [I, W])

Strategy (8 NeuronCores, SPMD).  SWDGE descriptor generation on the Pool
engine (one Q7 core pair, ~1.5ns/descriptor + ~1us/instruction) is the
serial bottleneck, so the host works to minimize descriptor count:

  * rm and fm are 0/1-valued; only ~1/4 of the (o, k) tokens have a
    nonzero coefficient c[o,k] = rm[o,k] * s[idx[o,k], k].  Zero tokens
    are dropped host-side.
  * Outputs are assigned to cores by greedy graph coloring so outputs
    sharing an input land on different cores (an input used twice on
    one core forces two separate row fetches).
  * Each core gets its own gather table: a permutation of the x columns
    into 256B pair rows [x[:,ia] | x[:,ib]], paired so that both halves
    of a fetched row usually belong to the SAME output (one descriptor
    then covers two tokens).  Tokens whose input is already placed
    reuse that row.  ~42k descriptors/core vs 262k for the dense case.
  * Within each core outputs are sorted by row count so each 128-output
    block has near-uniform T; block position bi uses the max T over the
    8 cores, so all cores share one SPMD program.
  * Gathers are merged up to a 2048-descriptor cap per instruction
    (larger instructions stall on ring backpressure), rotating over the
    4 SWDGE queues.  DVE applies the per-half coefficients and reduces.
"""

import numpy as np

import concourse.bass as bass
import concourse.bacc as bacc
import concourse.mybir as mybir
from concourse.tile import TileContext
from concourse.bass_utils import run_bass_kernel_spmd
from concourse.library_config import mlp

I = 65536
O = 65536
W = 32
B = 32
NCORES = 8
NROWS = I // 2                # 32768 table rows (int16 gather index limit)
NBLK = (O // NCORES) // 128   # 64 block positions per core
DESC_CAP = 2048               # max descriptors per merged gather
NQ = 4                        # SWDGE queues used round-robin
F32 = mybir.dt.float32
I16 = mybir.dt.int16


def _build_nc(t_list, groups):
    sum_t = sum(t_list)
    t_max = max(t_list)
    cap_t = max(sum(t_list[a:b]) for a, b in groups)
    nc = bacc.Bacc("TRN2", target_bir_lowering=False, debug=False,
                   num_devices=NCORES, num_swdge_queues=NQ,
                   dynamic_dma_scratch_size=65536)

    tab_d = nc.dram_tensor("tab", [NROWS, 2 * B], F32, kind="ExternalInput")
    idx_d = nc.dram_tensor("idx", [128, 8 * sum_t], I16, kind="ExternalInput")
    c01_d = nc.dram_tensor("c01", [128, 2 * sum_t], F32, kind="ExternalInput")
    out_d = nc.dram_tensor("out", [128, NBLK * B], F32, kind="ExternalOutput")

    with TileContext(nc) as tc:
        nc.gpsimd.load_library(mlp)

        with (
            tc.tile_pool(name="pres", bufs=1) as pres,
            tc.tile_pool(name="pg", bufs=6) as pg,
            tc.tile_pool(name="ptmp", bufs=3) as ptmp,
        ):
            idx_all = pres.tile([128, 8 * sum_t], I16)
            nc.sync.dma_start(idx_all[:], idx_d[:])
            c01_all = pres.tile([128, 2 * sum_t], F32)
            nc.sync.dma_start(c01_all[:], c01_d[:])
            ocore = pres.tile([128, NBLK * B], F32)

            goff = [0]
            for a, b in groups:
                goff.append(goff[-1] + sum(t_list[a:b]))

            for g, (a, bnd) in enumerate(groups):
                tg = sum(t_list[a:bnd])
                off = goff[g]
                if tg == 0:
                    for bi in range(a, bnd):
                        nc.vector.memset(
                            ocore[:, bi * B:(bi + 1) * B], 0.0)
                    continue
                G = pg.tile([128, cap_t, 2 * B], F32, tag="G")
                nc.gpsimd.dma_gather(
                    G[:, :tg, :], tab_d[:, :],
                    idx_all[:, 8 * off:8 * (off + tg)],
                    128 * tg, 128 * tg, 2 * B,
                    single_packet=False, queue_num=g % NQ)

                gv = G[:]
                boff = 0
                for bi in range(a, bnd):
                    T = t_list[bi]
                    osl = ocore[:, bi * B:(bi + 1) * B]
                    if T == 0:
                        nc.vector.memset(osl, 0.0)
                        continue
                    cv = c01_all[:, 2 * (off + boff):2 * (off + boff + T)]
                    tmp = ptmp.tile([128, B, 2 * t_max], F32, tag="tmp")
                    tv = tmp[:]
                    # tmp[p, b, u] = G[p, boff*64 + 32u + b] * c01[p, u]
                    gx = bass.AP(gv.tensor, gv.offset + boff * 2 * B,
                                 [list(gv.ap[0]), [B, 2 * T], [1, B]])
                    ab = bass.AP(cv.tensor, cv.offset,
                                 [list(cv.ap[0]), [1, 2 * T], [0, B]])
                    t_ap = bass.AP(tv.tensor, tv.offset,
                                   [list(tv.ap[0]), [1, 2 * T],
                                    [2 * t_max, B]])
                    nc.vector.tensor_mul(t_ap, gx, ab)

                    red_in = bass.AP(tv.tensor, tv.offset,
                                     [list(tv.ap[0]), [2 * t_max, B],
                                      [1, 2 * T]])
                    nc.vector.reduce_sum(osl, red_in,
                                         axis=mybir.AxisListType.X)
                    boff += T

            nc.sync.dma_start(out_d[:], ocore[:])

    nc.compile()
    return nc


def make_plan(x, forward_weights, forward_mask, output_mapping, reverse_mask):
    """Host-side planning: token extraction, core coloring, row pairing."""
    idx = np.asarray(output_mapping).astype(np.int64)
    rm = np.asarray(reverse_mask, dtype=np.float32)
    s = (np.asarray(forward_weights, dtype=np.float32)
         * np.asarray(forward_mask, dtype=np.float32))
    cols = np.arange(W)[None, :]
    c = rm * s[idx, cols]                                  # [O, W]
    nz = c != 0
    cnt = nz.sum(1)
    order = np.argsort(-cnt, kind="stable").tolist()

    # per-output token lists: (input, coeff) with duplicates aggregated
    toks = [None] * O
    for o in range(O):
        k = np.nonzero(nz[o])[0]
        ii = idx[o][k]
        cc = c[o][k]
        if len(ii) != len(set(ii.tolist())):
            agg = {}
            for i, cv in zip(ii.tolist(), cc.tolist()):
                agg[i] = agg.get(i, 0.0) + cv
            toks[o] = list(agg.items())
        else:
            toks[o] = list(zip(ii.tolist(), cc.tolist()))

    # ---- coloring: outputs sharing an input go to different cores
    input_mask = [0] * I
    core_load = [0] * NCORES
    cap = O // NCORES
    core_outputs = [[] for _ in range(NCORES)]
    for o in order:
        forb = 0
        for i, _ in toks[o]:
            forb |= input_mask[i]
        best, bestload = -1, 1 << 30
        for cc_ in range(NCORES):
            if core_load[cc_] >= cap or (forb >> cc_) & 1:
                continue
            if core_load[cc_] < bestload:
                best, bestload = cc_, core_load[cc_]
        if best < 0:
            bestkey = (1 << 30, 1 << 30)
            for cc_ in range(NCORES):
                if core_load[cc_] >= cap:
                    continue
                nconf = sum((input_mask[i] >> cc_) & 1 for i, _ in toks[o])
                key = (nconf, core_load[cc_])
                if key < bestkey:
                    bestkey, best = key, cc_
        core_outputs[best].append(o)
        core_load[best] += 1
        for i, _ in toks[o]:
            input_mask[i] |= 1 << best

    # ---- per-core greedy pairing into 256B rows
    plans = []
    for core in range(NCORES):
        placed = {}            # input -> (row, half)
        row_free = {}          # row -> free half
        nrows = 0
        slots = {}             # output -> list of [row, c0, c1]
        for o in core_outputs[core]:
            free = []
            touched = {}       # row -> [row, c0, c1]
            for i, cv in toks[o]:
                p = placed.get(i)
                if p is None:
                    free.append((i, cv))
                else:
                    sl = touched.get(p[0])
                    if sl is None:
                        sl = touched[p[0]] = [p[0], 0.0, 0.0]
                    sl[1 + p[1]] += cv
            nf = []
            for i, cv in free:
                done = False
                for r in touched:
                    h = row_free.pop(r, None)
                    if h is not None:
                        placed[i] = (r, h)
                        touched[r][1 + h] += cv
                        done = True
                        break
                if not done:
                    nf.append((i, cv))
            free = nf
            for g in range(len(free) // 2):
                (ia, ca), (ib, cb) = free[2 * g], free[2 * g + 1]
                if nrows >= NROWS:
                    raise RuntimeError("row overflow")
                placed[ia] = (nrows, 0)
                placed[ib] = (nrows, 1)
                touched[nrows] = [nrows, ca, cb]
                nrows += 1
            if len(free) % 2:
                i, cv = free[-1]
                r = None
                for rr in row_free:
                    if rr not in touched:
                        r = rr
                        break
                if r is not None:
                    h = row_free.pop(r)
                    placed[i] = (r, h)
                    sl = [r, 0.0, 0.0]
                    sl[1 + h] = cv
                    touched[r] = sl
                else:
                    if nrows >= NROWS:
                        raise RuntimeError("row overflow")
                    placed[i] = (nrows, 0)
                    row_free[nrows] = 1
                    touched[nrows] = [nrows, cv, 0.0]
                    nrows += 1
            slots[o] = list(touched.values())
        # row -> input map for the table
        row_inputs = np.zeros((NROWS, 2), np.int64)
        for i, (r, h) in placed.items():
            row_inputs[r, h] = i
        # sort outputs by slot count desc for uniform blocks
        ordered = sorted(core_outputs[core],
                         key=lambda o: -len(slots[o]))
        plans.append({"slots": slots, "ordered": ordered,
                      "row_inputs": row_inputs})

    # shared t_list across cores
    t_list = []
    for bi in range(NBLK):
        t = 0
        for pl in plans:
            blk = pl["ordered"][bi * 128:(bi + 1) * 128]
            t = max(t, max(len(pl["slots"][o]) for o in blk))
        t_list.append(t)
    t_list = tuple(t_list)

    # merge consecutive blocks into gathers of <= DESC_CAP descriptors
    groups = []
    a = 0
    while a < NBLK:
        b = a + 1
        tg = t_list[a]
        while b < NBLK and (tg + t_list[b]) * 128 <= DESC_CAP:
            tg += t_list[b]
            b += 1
        groups.append((a, b))
        a = b
    groups = tuple(groups)

    return {"plans": plans, "t_list": t_list, "groups": groups}


def make_in_maps(x, plan):
    x = np.asarray(x, dtype=np.float32)
    xT = np.ascontiguousarray(x.T)                         # [I, B]
    t_list = plan["t_list"]

    in_maps = []
    for core in range(NCORES):
        pl = plan["plans"][core]
        slots, ordered, row_inputs = (
            pl["slots"], pl["ordered"], pl["row_inputs"])
        tab = xT[row_inputs.reshape(-1)].reshape(NROWS, 2 * B)
        idx_parts, c_parts = [], []
        for bi, T in enumerate(t_list):
            outs = ordered[bi * 128:(bi + 1) * 128]
            if T == 0:
                continue
            J = np.zeros((128, T), np.int16)
            c01 = np.zeros((128, T, 2), np.float32)
            for p, o in enumerate(outs):
                for s_, (r, c0, c1) in enumerate(slots[o]):
                    J[p, s_] = r
                    c01[p, s_, 0] = c0
                    c01[p, s_, 1] = c1
            # token m = s*128 + p; wrap in 16 partitions, replicate x8
            L = J.T.reshape(8 * T, 16).T                   # [16, 8T]
            idx_parts.append(np.tile(L, (8, 1)))           # [128, 8T]
            c_parts.append(c01.reshape(128, 2 * T))
        in_maps.append({
            "tab": tab,
            "idx": np.ascontiguousarray(np.concatenate(idx_parts, axis=1)),
            "c01": np.ascontiguousarray(np.concatenate(c_parts, axis=1)),
        })
    return in_maps


def unshard_out(results, plan):
    out = np.empty((B, O), np.float32)
    for core in range(NCORES):
        oc = results[core]["out"]                          # [128, NBLK*B]
        vals = oc.reshape(128, NBLK, B).transpose(2, 1, 0) # [B, NBLK, 128]
        perm = np.array(plan["plans"][core]["ordered"])
        out[:, perm] = vals.reshape(B, NBLK * 128)
    return out


_NC = None
_NC_KEY = None


def _get_nc(t_list, groups):
    global _NC, _NC_KEY
    key = (t_list, groups)
    if _NC is None or _NC_KEY != key:
        _NC = _build_nc(t_list, groups)
        _NC_KEY = key
    return _NC


def kernel(x, forward_weights, forward_mask, output_mapping, reverse_mask):
    plan = make_plan(x, forward_weights, forward_mask,
                     output_mapping, reverse_mask)
    nc = _get_nc(plan["t_list"], plan["groups"])
    in_maps = make_in_maps(x, plan)
    res = run_bass_kernel_spmd(nc, in_maps, core_ids=list(range(NCORES)))
    return unshard_out(res.results, plan)


# revision 9
# speedup vs baseline: 1.1050x; 1.0089x over previous
"""Trainium2 Bass kernel for nn_DenSparseMatrix (gnn_message_passing).

v6: as v4 (pair-permutation table, coloring, capped merged gathers) plus
  * group-uniform T padding: one DVE multiply and one segmented reduce
    per gather group instead of per block (vector instruction overhead
    was ~50% of DVE busy time),
  * a tiny warmup gather to absorb the first-instruction Q7 cold cost,
  * single_packet=True (256B descriptors benefit from packet concat).
"""

import numpy as np

import concourse.bass as bass
import concourse.bacc as bacc
import concourse.mybir as mybir
from concourse.tile import TileContext
from concourse.bass_utils import run_bass_kernel_spmd
from concourse.library_config import mlp

I = 65536
O = 65536
W = 32
B = 32
NCORES = 8
NROWS = I // 2                # 32768 table rows (int16 gather index limit)
NBLK = (O // NCORES) // 128   # 64 block positions per core
DESC_CAP = 2048               # max descriptors per merged gather
NQ = 4                        # SWDGE queues used round-robin
F32 = mybir.dt.float32
I16 = mybir.dt.int16
SINGLE_PACKET = False


def _build_nc(t_list, groups):
    # groups: tuple of (a, b, tg) with uniform padded T = tg per block
    sum_u = sum((b - a) * tg for a, b, tg in groups)
    cap_u = max((b - a) * tg for a, b, tg in groups)
    nc = bacc.Bacc("TRN2", target_bir_lowering=False, debug=False,
                   num_devices=NCORES, num_swdge_queues=NQ)

    tab_d = nc.dram_tensor("tab", [NROWS, 2 * B], F32, kind="ExternalInput")
    idx_d = nc.dram_tensor("idx", [128, 8 * (sum_u + 1)], I16,
                           kind="ExternalInput")
    c01_d = nc.dram_tensor("c01", [128, 2 * sum_u], F32, kind="ExternalInput")
    out_d = nc.dram_tensor("out", [128, NBLK * B], F32, kind="ExternalOutput")

    with TileContext(nc) as tc:
        nc.gpsimd.load_library(mlp)

        with (
            tc.tile_pool(name="pres", bufs=1) as pres,
            tc.tile_pool(name="pg", bufs=6) as pg,
            tc.tile_pool(name="ptmp", bufs=3) as ptmp,
        ):
            idx_all = pres.tile([128, 8 * (sum_u + 1)], I16)
            nc.sync.dma_start(idx_all[:], idx_d[:])
            c01_all = pres.tile([128, 2 * sum_u], F32)
            nc.sync.dma_start(c01_all[:], c01_d[:])
            ocore = pres.tile([128, NBLK * B], F32)

            # warmup gather: 128 idxs (wrapped at the tail of idx_all)
            Gw = pg.tile([128, cap_u, 2 * B], F32, tag="G")
            nc.gpsimd.dma_gather(
                Gw[:, :1, :], tab_d[:, :],
                idx_all[:, 8 * sum_u:8 * (sum_u + 1)],
                128, 128, 2 * B,
                single_packet=SINGLE_PACKET, queue_num=0)

            off = 0
            ooff = 0
            for g, (a, bnd, tg) in enumerate(groups):
                nb = bnd - a
                u = nb * 2 * tg
                if tg == 0:
                    nc.vector.memset(ocore[:, ooff:ooff + nb * B], 0.0)
                    ooff += nb * B
                    continue
                G = pg.tile([128, cap_u, 2 * B], F32, tag="G")
                nc.gpsimd.dma_gather(
                    G[:, :nb * tg, :], tab_d[:, :],
                    idx_all[:, 8 * off:8 * (off + nb * tg)],
                    128 * nb * tg, 128 * nb * tg, 2 * B,
                    single_packet=SINGLE_PACKET, queue_num=g % NQ)

                gv = G[:]
                cv = c01_all[:, 2 * off:2 * (off + nb * tg)]
                tmp = ptmp.tile([128, B, 2 * cap_u], F32, tag="tmp")
                tv = tmp[:]
                # tmp[p, b, u] = G[p, 32u + b] * c01[p, u]
                gx = bass.AP(gv.tensor, gv.offset,
                             [list(gv.ap[0]), [B, u], [1, B]])
                ab = bass.AP(cv.tensor, cv.offset,
                             [list(cv.ap[0]), [1, u], [0, B]])
                t_ap = bass.AP(tv.tensor, tv.offset,
                               [list(tv.ap[0]), [1, u], [2 * cap_u, B]])
                nc.vector.tensor_mul(t_ap, gx, ab)

                # out[p, b, j] = sum_u tmp[p, b, j*2tg + u']
                red_in = bass.AP(tv.tensor, tv.offset,
                                 [list(tv.ap[0]), [2 * cap_u, B],
                                  [2 * tg, nb], [1, 2 * tg]])
                osl = ocore[:, ooff:ooff + nb * B]
                nc.vector.reduce_sum(osl, red_in, axis=mybir.AxisListType.X)
                off += nb * tg
                ooff += nb * B

            nc.sync.dma_start(out_d[:], ocore[:])

    nc.compile()
    return nc


def make_plan(x, forward_weights, forward_mask, output_mapping, reverse_mask):
    """Host-side planning: token extraction, core coloring, row pairing."""
    idx = np.asarray(output_mapping).astype(np.int64)
    rm = np.asarray(reverse_mask, dtype=np.float32)
    s = (np.asarray(forward_weights, dtype=np.float32)
         * np.asarray(forward_mask, dtype=np.float32))
    cols = np.arange(W)[None, :]
    c = rm * s[idx, cols]                                  # [O, W]
    nz = c != 0
    cnt = nz.sum(1)
    order = np.argsort(-cnt, kind="stable").tolist()

    toks = [None] * O
    for o in range(O):
        k = np.nonzero(nz[o])[0]
        ii = idx[o][k]
        cc = c[o][k]
        if len(ii) != len(set(ii.tolist())):
            agg = {}
            for i, cv in zip(ii.tolist(), cc.tolist()):
                agg[i] = agg.get(i, 0.0) + cv
            toks[o] = list(agg.items())
        else:
            toks[o] = list(zip(ii.tolist(), cc.tolist()))

    # ---- coloring: outputs sharing an input go to different cores
    input_mask = [0] * I
    core_load = [0] * NCORES
    cap = O // NCORES
    core_outputs = [[] for _ in range(NCORES)]
    for o in order:
        forb = 0
        for i, _ in toks[o]:
            forb |= input_mask[i]
        best, bestload = -1, 1 << 30
        for cc_ in range(NCORES):
            if core_load[cc_] >= cap or (forb >> cc_) & 1:
                continue
            if core_load[cc_] < bestload:
                best, bestload = cc_, core_load[cc_]
        if best < 0:
            bestkey = (1 << 30, 1 << 30)
            for cc_ in range(NCORES):
                if core_load[cc_] >= cap:
                    continue
                nconf = sum((input_mask[i] >> cc_) & 1 for i, _ in toks[o])
                key = (nconf, core_load[cc_])
                if key < bestkey:
                    bestkey, best = key, cc_
        core_outputs[best].append(o)
        core_load[best] += 1
        for i, _ in toks[o]:
            input_mask[i] |= 1 << best

    # ---- per-core greedy pairing into 256B rows
    plans = []
    for core in range(NCORES):
        placed = {}
        row_free = {}
        nrows = 0
        slots = {}
        for o in core_outputs[core]:
            free = []
            touched = {}
            for i, cv in toks[o]:
                p = placed.get(i)
                if p is None:
                    free.append((i, cv))
                else:
                    sl = touched.get(p[0])
                    if sl is None:
                        sl = touched[p[0]] = [p[0], 0.0, 0.0]
                    sl[1 + p[1]] += cv
            nf = []
            for i, cv in free:
                done = False
                for r in touched:
                    h = row_free.pop(r, None)
                    if h is not None:
                        placed[i] = (r, h)
                        touched[r][1 + h] += cv
                        done = True
                        break
                if not done:
                    nf.append((i, cv))
            free = nf
            for g in range(len(free) // 2):
                (ia, ca), (ib, cb) = free[2 * g], free[2 * g + 1]
                if nrows >= NROWS:
                    raise RuntimeError("row overflow")
                placed[ia] = (nrows, 0)
                placed[ib] = (nrows, 1)
                touched[nrows] = [nrows, ca, cb]
                nrows += 1
            if len(free) % 2:
                i, cv = free[-1]
                r = None
                for rr in row_free:
                    if rr not in touched:
                        r = rr
                        break
                if r is not None:
                    h = row_free.pop(r)
                    placed[i] = (r, h)
                    sl = [r, 0.0, 0.0]
                    sl[1 + h] = cv
                    touched[r] = sl
                else:
                    if nrows >= NROWS:
                        raise RuntimeError("row overflow")
                    placed[i] = (nrows, 0)
                    row_free[nrows] = 1
                    touched[nrows] = [nrows, cv, 0.0]
                    nrows += 1
            slots[o] = list(touched.values())
        row_inputs = np.zeros((NROWS, 2), np.int64)
        for i, (r, h) in placed.items():
            row_inputs[r, h] = i
        ordered = sorted(core_outputs[core],
                         key=lambda o: -len(slots[o]))
        plans.append({"slots": slots, "ordered": ordered,
                      "row_inputs": row_inputs})

    # shared t_list across cores
    t_list = []
    for bi in range(NBLK):
        t = 0
        for pl in plans:
            blk = pl["ordered"][bi * 128:(bi + 1) * 128]
            t = max(t, max(len(pl["slots"][o]) for o in blk))
        t_list.append(t)

    # merge consecutive blocks into uniform-T groups (<= DESC_CAP descs,
    # per-block padding waste <= 2 slots)
    groups = []
    a = 0
    while a < NBLK:
        tg = t_list[a]
        b = a + 1
        while (b < NBLK and (b - a + 1) * tg * 128 <= DESC_CAP
               and t_list[b] >= tg - 2):
            b += 1
        groups.append((a, b, tg))
        a = b
    groups = tuple(groups)

    return {"plans": plans, "t_list": tuple(t_list), "groups": groups}


def make_in_maps(x, plan):
    x = np.asarray(x, dtype=np.float32)
    xT = np.ascontiguousarray(x.T)                         # [I, B]
    groups = plan["groups"]

    in_maps = []
    for core in range(NCORES):
        pl = plan["plans"][core]
        slots, ordered, row_inputs = (
            pl["slots"], pl["ordered"], pl["row_inputs"])
        tab = xT[row_inputs.reshape(-1)].reshape(NROWS, 2 * B)
        idx_parts, c_parts = [], []
        for a, bnd, T in groups:
            for bi in range(a, bnd):
                outs = ordered[bi * 128:(bi + 1) * 128]
                if T == 0:
                    continue
                J = np.zeros((128, T), np.int16)
                c01 = np.zeros((128, T, 2), np.float32)
                for p, o in enumerate(outs):
                    for s_, (r, c0, c1) in enumerate(slots[o]):
                        J[p, s_] = r
                        c01[p, s_, 0] = c0
                        c01[p, s_, 1] = c1
                L = J.T.reshape(8 * T, 16).T               # [16, 8T]
                idx_parts.append(np.tile(L, (8, 1)))       # [128, 8T]
                c_parts.append(c01.reshape(128, 2 * T))
        idx_parts.append(np.zeros((128, 8), np.int16))     # warmup idxs
        in_maps.append({
            "tab": tab,
            "idx": np.ascontiguousarray(np.concatenate(idx_parts, axis=1)),
            "c01": np.ascontiguousarray(np.concatenate(c_parts, axis=1)),
        })
    return in_maps


def unshard_out(results, plan):
    groups = plan["groups"]
    out = np.empty((B, O), np.float32)
    for core in range(NCORES):
        oc = results[core]["out"]                          # [128, NBLK*B]
        ordered = np.array(plan["plans"][core]["ordered"])
        ooff = 0
        for a, bnd, T in groups:
            nb = bnd - a
            vals = oc[:, ooff:ooff + nb * B].reshape(128, B, nb)
            perm = ordered[a * 128:bnd * 128].reshape(nb, 128)
            for j in range(nb):
                out[:, perm[j]] = vals[:, :, j].T
            ooff += nb * B
    return out


_NC = None
_NC_KEY = None


def _get_nc(t_list, groups):
    global _NC, _NC_KEY
    key = (t_list, groups)
    if _NC is None or _NC_KEY != key:
        _NC = _build_nc(t_list, groups)
        _NC_KEY = key
    return _NC


def kernel(x, forward_weights, forward_mask, output_mapping, reverse_mask):
    plan = make_plan(x, forward_weights, forward_mask,
                     output_mapping, reverse_mask)
    nc = _get_nc(plan["t_list"], plan["groups"])
    in_maps = make_in_maps(x, plan)
    res = run_bass_kernel_spmd(nc, in_maps, core_ids=list(range(NCORES)))
    return unshard_out(res.results, plan)


# revision 11
# speedup vs baseline: 1.1119x; 1.0062x over previous
"""Trainium2 Bass kernel for nn_DenSparseMatrix (gnn_message_passing).

v6: as v4 (pair-permutation table, coloring, capped merged gathers) plus
  * group-uniform T padding: one DVE multiply and one segmented reduce
    per gather group instead of per block (vector instruction overhead
    was ~50% of DVE busy time),
  * a tiny warmup gather to absorb the first-instruction Q7 cold cost,
  * single_packet=True (256B descriptors benefit from packet concat).
"""

import numpy as np

import concourse.bass as bass
import concourse.bacc as bacc
import concourse.mybir as mybir
from concourse.tile import TileContext
from concourse.bass_utils import run_bass_kernel_spmd
from concourse.library_config import mlp

I = 65536
O = 65536
W = 32
B = 32
NCORES = 8
NROWS = I // 2                # 32768 table rows (int16 gather index limit)
NBLK = (O // NCORES) // 128   # 64 block positions per core
DESC_CAP = 2048               # max descriptors per merged gather
NQ = 4                        # SWDGE queues used round-robin
F32 = mybir.dt.float32
I16 = mybir.dt.int16
SINGLE_PACKET = False


SPLIT = 6                     # groups served by the small head chunk


def _build_nc(t_list, groups):
    # groups: tuple of (a, b, tg) with uniform padded T = tg per block
    usz = [(b - a) * tg for a, b, tg in groups]
    sum_u = sum(usz)
    cap_u = max(usz)
    u1 = sum(usz[:SPLIT])     # tokens in head chunk
    nc = bacc.Bacc("TRN2", target_bir_lowering=False, debug=False,
                   num_devices=NCORES, num_swdge_queues=NQ,
                   dynamic_dma_scratch_size=65536)

    tab_d = nc.dram_tensor("tab", [NROWS, 2 * B], F32, kind="ExternalInput")
    idxw_d = nc.dram_tensor("idxw", [128, 8], I16, kind="ExternalInput")
    idx1_d = nc.dram_tensor("idx1", [128, 8 * u1], I16, kind="ExternalInput")
    c011_d = nc.dram_tensor("c011", [128, 2 * u1], F32, kind="ExternalInput")
    idx2_d = nc.dram_tensor("idx2", [128, 8 * (sum_u - u1)], I16,
                            kind="ExternalInput")
    c012_d = nc.dram_tensor("c012", [128, 2 * (sum_u - u1)], F32,
                            kind="ExternalInput")
    out_d = nc.dram_tensor("out", [128, NBLK * B], F32, kind="ExternalOutput")

    with TileContext(nc) as tc:
        nc.gpsimd.load_library(mlp)

        with (
            tc.tile_pool(name="pres", bufs=1) as pres,
            tc.tile_pool(name="pg", bufs=6) as pg,
            tc.tile_pool(name="ptmp", bufs=3) as ptmp,
        ):
            idx_w = pres.tile([128, 8], I16)
            nc.sync.dma_start(idx_w[:], idxw_d[:])
            idx_1 = pres.tile([128, 8 * u1], I16)
            nc.sync.dma_start(idx_1[:], idx1_d[:])
            c01_1 = pres.tile([128, 2 * u1], F32)
            nc.sync.dma_start(c01_1[:], c011_d[:])
            idx_2 = pres.tile([128, 8 * (sum_u - u1)], I16)
            nc.sync.dma_start(idx_2[:], idx2_d[:])
            c01_2 = pres.tile([128, 2 * (sum_u - u1)], F32)
            nc.sync.dma_start(c01_2[:], c012_d[:])
            ocore = pres.tile([128, NBLK * B], F32)

            # warmup gather: 128 idxs from its own tiny tile
            Gw = pg.tile([128, cap_u, 2 * B], F32, tag="G")
            nc.gpsimd.dma_gather(
                Gw[:, :1, :], tab_d[:, :], idx_w[:],
                128, 128, 2 * B,
                single_packet=SINGLE_PACKET, queue_num=0)

            off = 0
            ooff = 0
            for g, (a, bnd, tg) in enumerate(groups):
                nb = bnd - a
                u = nb * 2 * tg
                if tg == 0:
                    nc.vector.memset(ocore[:, ooff:ooff + nb * B], 0.0)
                    ooff += nb * B
                    continue
                if g < SPLIT:
                    idx_t, c01_t, coff = idx_1, c01_1, off
                else:
                    idx_t, c01_t, coff = idx_2, c01_2, off - u1
                G = pg.tile([128, cap_u, 2 * B], F32, tag="G")
                nc.gpsimd.dma_gather(
                    G[:, :nb * tg, :], tab_d[:, :],
                    idx_t[:, 8 * coff:8 * (coff + nb * tg)],
                    128 * nb * tg, 128 * nb * tg, 2 * B,
                    single_packet=SINGLE_PACKET, queue_num=g % NQ)

                gv = G[:]
                cv = c01_t[:, 2 * coff:2 * (coff + nb * tg)]
                tmp = ptmp.tile([128, B, 2 * cap_u], F32, tag="tmp")
                tv = tmp[:]
                # tmp[p, b, u] = G[p, 32u + b] * c01[p, u]
                gx = bass.AP(gv.tensor, gv.offset,
                             [list(gv.ap[0]), [B, u], [1, B]])
                ab = bass.AP(cv.tensor, cv.offset,
                             [list(cv.ap[0]), [1, u], [0, B]])
                t_ap = bass.AP(tv.tensor, tv.offset,
                               [list(tv.ap[0]), [1, u], [2 * cap_u, B]])
                nc.vector.tensor_mul(t_ap, gx, ab)

                # out[p, b, j] = sum_u tmp[p, b, j*2tg + u']
                red_in = bass.AP(tv.tensor, tv.offset,
                                 [list(tv.ap[0]), [2 * cap_u, B],
                                  [2 * tg, nb], [1, 2 * tg]])
                osl = ocore[:, ooff:ooff + nb * B]
                nc.vector.reduce_sum(osl, red_in, axis=mybir.AxisListType.X)
                off += nb * tg
                ooff += nb * B

            nc.sync.dma_start(out_d[:], ocore[:])

    nc.compile()
    return nc


def make_plan(x, forward_weights, forward_mask, output_mapping, reverse_mask):
    """Host-side planning: token extraction, core coloring, row pairing."""
    idx = np.asarray(output_mapping).astype(np.int64)
    rm = np.asarray(reverse_mask, dtype=np.float32)
    s = (np.asarray(forward_weights, dtype=np.float32)
         * np.asarray(forward_mask, dtype=np.float32))
    cols = np.arange(W)[None, :]
    c = rm * s[idx, cols]                                  # [O, W]
    nz = c != 0
    cnt = nz.sum(1)
    order = np.argsort(-cnt, kind="stable").tolist()

    toks = [None] * O
    for o in range(O):
        k = np.nonzero(nz[o])[0]
        ii = idx[o][k]
        cc = c[o][k]
        if len(ii) != len(set(ii.tolist())):
            agg = {}
            for i, cv in zip(ii.tolist(), cc.tolist()):
                agg[i] = agg.get(i, 0.0) + cv
            toks[o] = list(agg.items())
        else:
            toks[o] = list(zip(ii.tolist(), cc.tolist()))

    # ---- coloring: outputs sharing an input go to different cores
    input_mask = [0] * I
    core_load = [0] * NCORES
    cap = O // NCORES
    core_outputs = [[] for _ in range(NCORES)]
    for o in order:
        forb = 0
        for i, _ in toks[o]:
            forb |= input_mask[i]
        best, bestload = -1, 1 << 30
        for cc_ in range(NCORES):
            if core_load[cc_] >= cap or (forb >> cc_) & 1:
                continue
            if core_load[cc_] < bestload:
                best, bestload = cc_, core_load[cc_]
        if best < 0:
            bestkey = (1 << 30, 1 << 30)
            for cc_ in range(NCORES):
                if core_load[cc_] >= cap:
                    continue
                nconf = sum((input_mask[i] >> cc_) & 1 for i, _ in toks[o])
                key = (nconf, core_load[cc_])
                if key < bestkey:
                    bestkey, best = key, cc_
        core_outputs[best].append(o)
        core_load[best] += 1
        for i, _ in toks[o]:
            input_mask[i] |= 1 << best

    # ---- per-core greedy pairing into 256B rows
    plans = []
    for core in range(NCORES):
        placed = {}
        row_free = {}
        nrows = 0
        slots = {}
        for o in core_outputs[core]:
            free = []
            touched = {}
            for i, cv in toks[o]:
                p = placed.get(i)
                if p is None:
                    free.append((i, cv))
                else:
                    sl = touched.get(p[0])
                    if sl is None:
                        sl = touched[p[0]] = [p[0], 0.0, 0.0]
                    sl[1 + p[1]] += cv
            nf = []
            for i, cv in free:
                done = False
                for r in touched:
                    h = row_free.pop(r, None)
                    if h is not None:
                        placed[i] = (r, h)
                        touched[r][1 + h] += cv
                        done = True
                        break
                if not done:
                    nf.append((i, cv))
            free = nf
            for g in range(len(free) // 2):
                (ia, ca), (ib, cb) = free[2 * g], free[2 * g + 1]
                if nrows >= NROWS:
                    raise RuntimeError("row overflow")
                placed[ia] = (nrows, 0)
                placed[ib] = (nrows, 1)
                touched[nrows] = [nrows, ca, cb]
                nrows += 1
            if len(free) % 2:
                i, cv = free[-1]
                r = None
                for rr in row_free:
                    if rr not in touched:
                        r = rr
                        break
                if r is not None:
                    h = row_free.pop(r)
                    placed[i] = (r, h)
                    sl = [r, 0.0, 0.0]
                    sl[1 + h] = cv
                    touched[r] = sl
                else:
                    if nrows >= NROWS:
                        raise RuntimeError("row overflow")
                    placed[i] = (nrows, 0)
                    row_free[nrows] = 1
                    touched[nrows] = [nrows, cv, 0.0]
                    nrows += 1
            slots[o] = list(touched.values())
        row_inputs = np.zeros((NROWS, 2), np.int64)
        for i, (r, h) in placed.items():
            row_inputs[r, h] = i
        ordered = sorted(core_outputs[core],
                         key=lambda o: -len(slots[o]))
        plans.append({"slots": slots, "ordered": ordered,
                      "row_inputs": row_inputs})

    # shared t_list across cores
    t_list = []
    for bi in range(NBLK):
        t = 0
        for pl in plans:
            blk = pl["ordered"][bi * 128:(bi + 1) * 128]
            t = max(t, max(len(pl["slots"][o]) for o in blk))
        t_list.append(t)

    # merge consecutive blocks into uniform-T groups (<= DESC_CAP descs,
    # per-block padding waste <= 2 slots)
    groups = []
    a = 0
    while a < NBLK:
        tg = t_list[a]
        b = a + 1
        while (b < NBLK and (b - a + 1) * tg * 128 <= DESC_CAP
               and t_list[b] >= tg - 2):
            b += 1
        groups.append((a, b, tg))
        a = b
    groups = tuple(groups)

    return {"plans": plans, "t_list": tuple(t_list), "groups": groups}


def make_in_maps(x, plan):
    x = np.asarray(x, dtype=np.float32)
    xT = np.ascontiguousarray(x.T)                         # [I, B]
    groups = plan["groups"]

    in_maps = []
    for core in range(NCORES):
        pl = plan["plans"][core]
        slots, ordered, row_inputs = (
            pl["slots"], pl["ordered"], pl["row_inputs"])
        tab = xT[row_inputs.reshape(-1)].reshape(NROWS, 2 * B)
        idx_parts, c_parts = [], []
        nsplit = [0, 0]        # parts in head chunk (idx, c01)
        for gi, (a, bnd, T) in enumerate(groups):
            for bi in range(a, bnd):
                outs = ordered[bi * 128:(bi + 1) * 128]
                if T == 0:
                    continue
                J = np.zeros((128, T), np.int16)
                c01 = np.zeros((128, T, 2), np.float32)
                for p, o in enumerate(outs):
                    for s_, (r, c0, c1) in enumerate(slots[o]):
                        J[p, s_] = r
                        c01[p, s_, 0] = c0
                        c01[p, s_, 1] = c1
                L = J.T.reshape(8 * T, 16).T               # [16, 8T]
                idx_parts.append(np.tile(L, (8, 1)))       # [128, 8T]
                c_parts.append(c01.reshape(128, 2 * T))
                if gi < SPLIT:
                    nsplit = [len(idx_parts), len(c_parts)]
        ni, ncp = nsplit
        cat = lambda ps: np.ascontiguousarray(np.concatenate(ps, axis=1))
        in_maps.append({
            "tab": tab,
            "idxw": np.zeros((128, 8), np.int16),
            "idx1": cat(idx_parts[:ni]),
            "c011": cat(c_parts[:ncp]),
            "idx2": cat(idx_parts[ni:]),
            "c012": cat(c_parts[ncp:]),
        })
    return in_maps


def unshard_out(results, plan):
    groups = plan["groups"]
    out = np.empty((B, O), np.float32)
    for core in range(NCORES):
        oc = results[core]["out"]                          # [128, NBLK*B]
        ordered = np.array(plan["plans"][core]["ordered"])
        ooff = 0
        for a, bnd, T in groups:
            nb = bnd - a
            vals = oc[:, ooff:ooff + nb * B].reshape(128, B, nb)
            perm = ordered[a * 128:bnd * 128].reshape(nb, 128)
            for j in range(nb):
                out[:, perm[j]] = vals[:, :, j].T
            ooff += nb * B
    return out


_NC = None
_NC_KEY = None


def _get_nc(t_list, groups):
    global _NC, _NC_KEY
    key = (t_list, groups)
    if _NC is None or _NC_KEY != key:
        _NC = _build_nc(t_list, groups)
        _NC_KEY = key
    return _NC


def kernel(x, forward_weights, forward_mask, output_mapping, reverse_mask):
    plan = make_plan(x, forward_weights, forward_mask,
                     output_mapping, reverse_mask)
    nc = _get_nc(plan["t_list"], plan["groups"])
    in_maps = make_in_maps(x, plan)
    res = run_bass_kernel_spmd(nc, in_maps, core_ids=list(range(NCORES)))
    return unshard_out(res.results, plan)
